# revision 47
# baseline (speedup 1.0000x reference)
"""GQA kernel for TRN2, 8 cores: DP2 (batch) x TP4 (KV-head pairs).

Core i: batch i//4, KV heads {2g, 2g+1} (g = i%4), Q heads 8g..8g+7.

Per core (B=1, T=2048, C=2048 local view):
  - All operands bf16 (host-converted); matmul accum fp32 in PSUM.
  - Q^T/K^T computed as [dims, tok] pair-tiles [128, 512]; RoPE on DVE
    (6 ops/tile) writes bf16 QT/KT. Q pairs are (q_j, q_j+4) so the two
    64-row halves use KV head A/B respectively (base-partition match).
  - V computed directly transposed: out[tok, vdim] (lhsT = x chunk).
  - Scores S^T[k, q] per 128-q subtile j, key tiles ki<=j only (fine
    causal granularity; no memsets). 8 heads share one [128, 1024] PSUM
    tile -> single exp per ki. Diagonal ki==j masked by one tri-multiply
    over all heads ([128,1024] with an 8x-replicated tri constant).
  - AV in [q, d] orientation: out[128 q, 64] per head, N=64 matmuls, plus
    N=1 ones-matmuls accumulating softmax denominators (PSUM [128, 8]).
    PSUM pending-zero is bank-granular, so each bank's accumulation group
    opens/closes exactly once per subtile.
  - Scores split in two 4-head groups of [128, 512] so PSUM fits
    (S 3-buf + AV + proj 2-buf + outproj + misc = 8 banks); all matmuls
    into one S tile share a tile_position (mixing 0/64 within one tile
    hangs the device).
  - Normalize: one DVE tensor_mul with a stride-0-broadcast reciprocal;
    transpose Y on PE into a bf16 view of the misc bank; out-projection
    accumulates [128, 512] C-chunks, copies spread over ACT/DVE.
  - Host sums the 4 bf16 partial outputs per batch.
"""

import sys

for p in ("/opt/trn_rl_repo", "/root/.axon_site/_ro/trn_rl_repo"):
    if p not in sys.path:
        sys.path.insert(0, p)

import numpy as np
import ml_dtypes
from contextlib import ExitStack

import concourse.bacc as bacc
import concourse.mybir as mybir
import concourse.tile as tile

F32 = mybir.dt.float32
BF16 = mybir.dt.bfloat16
BF16_NP = ml_dtypes.bfloat16

D = 64
ROPE_BASE = 10000.0
TCH = 512            # projection token chunk
EXPF = mybir.ActivationFunctionType.Exp
DEBUG_DUMP = False   # dump j=1 intermediates into `out` instead of results


def build_nc(C, T):
    CT = C // 128            # contraction tiles (16)
    NCH = T // TCH           # proj chunks (4)
    NJ = T // 128            # 128-token subtiles (16)
    JPC = TCH // 128         # subtiles per chunk (4)

    nc = bacc.Bacc("TRN2", target_bir_lowering=False, debug=False)

    xT = nc.dram_tensor("xT", [128, CT * T], BF16, kind="ExternalInput")
    wq = nc.dram_tensor("wq", [128, CT * 512], BF16, kind="ExternalInput")
    wkv = nc.dram_tensor("wkv", [128, CT * 256], BF16, kind="ExternalInput")
    wo = nc.dram_tensor("wo", [128, 4 * C], BF16, kind="ExternalInput")
    rqc = nc.dram_tensor("rqc", [128, T], BF16, kind="ExternalInput")
    rqs = nc.dram_tensor("rqs", [128, T], BF16, kind="ExternalInput")
    rkc = nc.dram_tensor("rkc", [128, T], BF16, kind="ExternalInput")
    rks = nc.dram_tensor("rks", [128, T], BF16, kind="ExternalInput")
    tri8 = nc.dram_tensor("tri8", [128, 1024], BF16, kind="ExternalInput")
    ident = nc.dram_tensor("ident", [128, 128], BF16, kind="ExternalInput")
    out = nc.dram_tensor("out", [T, C], BF16, kind="ExternalOutput")

    with tile.TileContext(nc) as tc, ExitStack() as ctx:
        # PSUM pools: creation order fixes bank layout; keep every pool's
        # per-partition size a multiple of 2048B so tiles stay bank-aligned.
        ps_s = ctx.enter_context(tc.tile_pool(name="ps_s", bufs=3, space="PSUM"))
        # AV accumulators and out-proj tiles share one 2-slot pool (same
        # tag): the AVP(j), OPS(j,0..3) allocation sequence ping-pongs OPS
        # units across slots (no serial wait on one bank), and every WAR
        # pairing is already ordered by existing dataflow deps.
        ps_w = ctx.enter_context(tc.tile_pool(name="ps_w", bufs=2, space="PSUM"))
        ps_pj = ctx.enter_context(tc.tile_pool(name="ps_pj", bufs=2, space="PSUM"))
        ps_m = ctx.enter_context(tc.tile_pool(name="ps_m", bufs=1, space="PSUM"))

        cst = ctx.enter_context(tc.tile_pool(name="cst", bufs=1))
        xcp = ctx.enter_context(tc.tile_pool(name="xcp", bufs=3))
        tmpp = ctx.enter_context(tc.tile_pool(name="tmpp", bufs=3))
        pp = ctx.enter_context(tc.tile_pool(name="pp", bufs=6))
        rcp = ctx.enter_context(tc.tile_pool(name="rcp", bufs=2))
        ysp = ctx.enter_context(tc.tile_pool(name="ysp", bufs=3))
        ytp = ctx.enter_context(tc.tile_pool(name="ytp", bufs=3))
        osp = ctx.enter_context(tc.tile_pool(name="osp", bufs=3))

        WQ = cst.tile([128, CT * 512], BF16, tag="WQ")
        WKV = cst.tile([128, CT * 256], BF16, tag="WKV")
        WO = cst.tile([128, 4 * C], BF16, tag="WO")
        QC = cst.tile([128, T], BF16, tag="QC")
        QS = cst.tile([128, T], BF16, tag="QS")
        KC = cst.tile([128, T], BF16, tag="KC")
        KS = cst.tile([128, T], BF16, tag="KS")
        TRI8 = cst.tile([128, 1024], BF16, tag="TRI8")
        ID = cst.tile([128, 128], BF16, tag="ID")
        ONES = cst.tile([128, 1], BF16, tag="ONES")
        QT = cst.tile([128, 4 * T], BF16, tag="QT")
        KT = cst.tile([128, T], BF16, tag="KT")
        VPA = cst.tile([128, NJ * 64], BF16, tag="VPA")
        VPB = cst.tile([128, NJ * 64], BF16, tag="VPB")
        # Persistent 1-bank PSUM scratch: softmax denominators (j-parity
        # rotated 8-col slots) + transpose staging (bf16 view of cols 16:272).
        MISC = ps_m.tile([128, 512], F32, tag="m")

        # DMA issue order sets DMA-device service order: first x half-chunk
        # and the weights that unblock the first projections, then tables,
        # then the rest.
        XCs = [xcp.tile([128, CT * TCH], BF16, tag="XC", name=f"XC{i}") for i in range(NCH)]
        QUAR = CT * TCH // 4
        # One need-ordered stream on SP (interleaving WKV/x quarters so the
        # K projection can start ~2us in), then the rest.
        WKVQ = CT * 256 // 4
        WQQ = CT * 512 // 4
        for qq in range(4):
            nc.sync.dma_start(WKV[:, qq * WKVQ:(qq + 1) * WKVQ],
                              wkv[:, qq * WKVQ:(qq + 1) * WKVQ])
            nc.sync.dma_start(XCs[0][:, qq * QUAR:(qq + 1) * QUAR],
                              xT[:, qq * QUAR:(qq + 1) * QUAR])
        nc.sync.dma_start(KC[:], rkc[:])
        nc.sync.dma_start(KS[:], rks[:])
        for qq in range(4):
            nc.sync.dma_start(WQ[:, qq * WQQ:(qq + 1) * WQQ],
                              wq[:, qq * WQQ:(qq + 1) * WQQ])
        nc.sync.dma_start(QC[:], rqc[:])
        nc.sync.dma_start(QS[:], rqs[:])
        nc.scalar.dma_start(TRI8[:], tri8[:])
        nc.scalar.dma_start(ID[:], ident[:])
        nc.sync.dma_start(XCs[1][:], xT[:, CT * TCH:2 * CT * TCH])
        nc.gpsimd.dma_start(WO[:], wo[:])
        nc.vector.memset(ONES[:], 1.0)

        def rope(ps, cosT, sinT, dst, tw0):
            """dst[128, 512] (bf16) = RoPE(ps[128, 512]) for a 2x64-row pair.
            One PSUM read (copy) frees the proj bank after ~0.7us instead of
            ~2.8us; the remaining ops are all-bf16 SBUF (2x DVE mode)."""
            psc = tmpp.tile([128, TCH], BF16, tag="psc")
            tco = tmpp.tile([128, TCH], BF16, tag="tco")
            tsi = tmpp.tile([128, TCH], BF16, tag="tsi")
            nc.vector.tensor_copy(psc[:], ps[:])
            nc.vector.tensor_mul(tco[:], psc[:], cosT[:, tw0:tw0 + TCH])
            for b0 in (0, 64):
                nc.vector.tensor_mul(
                    tsi[b0:b0 + 32, :], psc[b0 + 32:b0 + 64, :],
                    sinT[b0 + 32:b0 + 64, tw0:tw0 + TCH])
                nc.vector.tensor_mul(
                    tsi[b0 + 32:b0 + 64, :], psc[b0:b0 + 32, :],
                    sinT[b0:b0 + 32, tw0:tw0 + TCH])
            nc.vector.tensor_add(dst, tco[:], tsi[:])

        pend = {"work": None}

        def emit_outproj(n):
            """Emit up to n deferred out-projection co-units (4 MMs + copy),
            plus the output DMA when the unit completes."""
            w = pend["work"]
            if w is None:
                return
            j_, YT_, OSB_, co0 = w
            for co in range(co0, min(co0 + n, 4)):
                OPS = ps_w.tile([128, 512], F32, tag="w")
                for p in range(4):
                    nc.tensor.matmul(
                        OPS[:], YT_[:, p * 128:p * 128 + 128],
                        WO[:, p * C + co * 512:p * C + co * 512 + 512],
                        start=(p == 0), stop=(p == 3))
                if co == 0 and j_ < 8:
                    nc.scalar.copy(OSB_[:, co * 512:co * 512 + 512], OPS[:])
                else:
                    nc.vector.tensor_copy(OSB_[:, co * 512:co * 512 + 512], OPS[:])
            w[3] = min(co0 + n, 4)
            if w[3] == 4:
                nc.sync.dma_start(out[j_ * 128:(j_ + 1) * 128, :], OSB_[:])
                pend["work"] = None

        for tcl in range(NCH):
            XC = XCs[tcl]
            if tcl >= 2:
                nc.sync.dma_start(XC[:], xT[:, tcl * CT * TCH:(tcl + 1) * CT * TCH])
            tw0 = tcl * TCH
            # K projection (one [128,512] pair-tile: kv heads A/B)
            KPS = ps_pj.tile([128, TCH], F32, tag="pj")
            for ct in range(CT):
                nc.tensor.matmul(
                    KPS[:], WKV[:, ct * 256:ct * 256 + 128],
                    XC[:, ct * TCH:(ct + 1) * TCH],
                    start=(ct == 0), stop=(ct == CT - 1))
            rope(KPS, KC, KS, KT[:, tw0:tw0 + TCH], tw0)
            # Q projections (4 pair-tiles)
            for j in range(4):
                QPS = ps_pj.tile([128, TCH], F32, tag="pj")
                for ct in range(CT):
                    nc.tensor.matmul(
                        QPS[:], WQ[:, ct * 512 + j * 128:ct * 512 + j * 128 + 128],
                        XC[:, ct * TCH:(ct + 1) * TCH],
                        start=(ct == 0), stop=(ct == CT - 1))
                rope(QPS, QC, QS, QT[:, j * T + tw0:j * T + tw0 + TCH], tw0)
            # V projection: out[tok, vdim] per 128-token subtile
            VPS = ps_pj.tile([128, TCH], F32, tag="pj")
            for tt in range(JPC):
                for ct in range(CT):
                    nc.tensor.matmul(
                        VPS[:, tt * 128:tt * 128 + 128],
                        XC[:, ct * TCH + tt * 128:ct * TCH + tt * 128 + 128],
                        WKV[:, ct * 256 + 128:ct * 256 + 256],
                        start=(ct == 0), stop=(ct == CT - 1))
            for tt in range(JPC):
                kt = tcl * JPC + tt
                nc.vector.tensor_copy(VPA[:, kt * 64:kt * 64 + 64], VPS[:, tt * 128:tt * 128 + 64])
                nc.vector.tensor_copy(VPB[:, kt * 64:kt * 64 + 64], VPS[:, tt * 128 + 64:tt * 128 + 128])

            # attention for this chunk's four 128-q subtiles; out-projection
            # of subtile j is deferred and interleaved into attention j+1's
            # ki loop (fills PE during exp waits).
            for jj in range(JPC):
                j = tcl * JPC + jj
                AVP = ps_w.tile([128, 512], F32, tag="w")
                DEN = MISC[:, (j % 2) * 8:(j % 2) * 8 + 8]
                # Group A covers Q pairs 0-1 (heads 0,4,1,5), group B pairs
                # 2-3 -- so group A's whole ki loop can stream as soon as half
                # the Q projection of this chunk is done.
                # NOTE: start=True zeroes the whole 2KB PSUM bank (pending-
                # zero is bank-granular), so each bank's accumulation group
                # opens exactly once (first write) and closes on the last.
                for ki in range(j + 1):
                    for grp in range(2):
                        heads = [4 * grp, 4 * grp + 1, 4 * grp + 2, 4 * grp + 3]
                        S = ps_s.tile([128, 512], F32, tag="s")
                        for hh, h in enumerate(heads):
                            b0 = (h // 4) * 64
                            nc.tensor.matmul(
                                S[:, hh * 128:hh * 128 + 128],
                                KT[b0:b0 + 64, ki * 128:ki * 128 + 128],
                                QT[b0:b0 + 64, (h % 4) * T + j * 128:(h % 4) * T + j * 128 + 128],
                                start=True, stop=True)
                        P = pp.tile([128, 512], BF16, tag="P")
                        nc.scalar.activation(P[:], S[:], EXPF)
                        if ki == j:
                            nc.vector.tensor_mul(P[:], P[:], TRI8[:, 0:512])
                        for hh, h in enumerate(heads):
                            VP = VPA if h < 4 else VPB
                            nc.tensor.matmul(
                                AVP[:, h * 64:h * 64 + 64], P[:, hh * 128:hh * 128 + 128],
                                VP[:, ki * 64:ki * 64 + 64],
                                start=(grp == 0 and ki == 0 and hh == 0),
                                stop=(grp == 1 and ki == j and hh == 3),
                                skip_group_check=True)
                            nc.tensor.matmul(
                                DEN[:, h:h + 1], P[:, hh * 128:hh * 128 + 128], ONES[:],
                                start=(grp == 0 and ki == 0 and hh == 0),
                                stop=(grp == 1 and ki == j and hh == 3),
                                skip_group_check=True)

                RC = rcp.tile([128, 8], F32, tag="RC")
                nc.vector.reciprocal(RC[:], DEN)
                YSB = ysp.tile([128, 512], BF16, tag="YSB")
                nc.vector.tensor_mul(
                    YSB[:], AVP[:, 0:512],
                    RC[:, :, None].broadcast_to([128, 8, 64]))
                YTt = ytp.tile([128, 512], BF16, tag="YT")
                TPSv = MISC[:, 16:272].bitcast(BF16)
                for p in range(4):
                    nc.tensor.transpose(
                        TPSv[:, p * 128:p * 128 + 128],
                        YSB[:, p * 128:p * 128 + 128], ID[:])
                nc.vector.tensor_copy(YTt[:], TPSv[:])
                OSBn = osp.tile([128, C], BF16, tag="OSB", name=f"OSB{j}")
                pend["work"] = [j, YTt, OSBn, 0]
                emit_outproj(4)

        emit_outproj(4)

    nc.compile()
    return nc


def rope_tables(T, scale):
    inv = 1.0 / (ROPE_BASE ** (np.arange(0, D, 2, dtype=np.float32) / D))
    t = np.arange(T, dtype=np.float32)
    freqs = np.outer(t, inv)  # [T, 32]
    emb = np.concatenate([freqs, freqs], -1)  # [T, 64]
    cos = np.cos(emb).T.astype(np.float32) * scale  # [64, T]
    sin = np.sin(emb).T.astype(np.float32) * scale
    sinX = np.empty((64, T), np.float32)
    sinX[0:32] = sin[32:64]
    sinX[32:64] = -sin[0:32]
    cos2 = np.concatenate([cos, cos], 0)
    sin2 = np.concatenate([sinX, sinX], 0)
    return np.ascontiguousarray(cos2), np.ascontiguousarray(sin2)


def _img_ct(w, cols):
    """[C, cols] -> [128, CT*cols] image: img[p, ct*cols + c] = w[ct*128+p, c]."""
    C = w.shape[0]
    CT = C // 128
    return np.ascontiguousarray(
        w.reshape(CT, 128, cols).transpose(1, 0, 2).reshape(128, CT * cols))


def make_inputs(x, Wq, Wk, Wv, Wo):
    B, T, C = x.shape
    CT = C // 128
    qc, qs = rope_tables(T, 1.0 / np.sqrt(D).astype(np.float32))
    kc, ks = rope_tables(T, 1.0)
    tri = np.triu(np.ones((128, 128), np.float32)).astype(BF16_NP)
    tri8 = np.ascontiguousarray(np.tile(tri, (1, 8)))
    common = {
        "rqc": qc.astype(BF16_NP), "rqs": qs.astype(BF16_NP),
        "rkc": kc.astype(BF16_NP), "rks": ks.astype(BF16_NP),
        "tri8": tri8,
        "ident": np.eye(128, dtype=np.float32).astype(BF16_NP),
    }
    xT_img = []
    for b in range(B):
        # [128, NCH * CT * TCH]: chunk-major, then ct, then token
        xb = x[b].astype(BF16_NP)  # [T, C]
        img = (xb.reshape(T // TCH, TCH, CT, 128)
               .transpose(3, 0, 2, 1)   # [128, NCH, CT, TCH]
               .reshape(128, -1))
        xT_img.append(np.ascontiguousarray(img))
    in_maps = []
    for i in range(8):
        b, g = divmod(i, 4)
        # Q head pairing: pair j = (local head j, local head j+4)
        qcols = []
        for j in range(4):
            qcols.append(Wq[:, (8 * g + j) * 64:(8 * g + j) * 64 + 64])
            qcols.append(Wq[:, (8 * g + j + 4) * 64:(8 * g + j + 4) * 64 + 64])
        wq_p = np.concatenate(qcols, 1)  # [C, 512] pair-major
        wkv_p = np.concatenate(
            [Wk[:, (2 * g) * 64:(2 * g + 2) * 64],
             Wv[:, (2 * g) * 64:(2 * g + 2) * 64]], 1)  # [C, 256]
        # Y/YT dim order comes out as natural local heads 0..7, so Wo rows
        # are the plain local slice.
        wo_p = Wo[8 * g * 64:(8 * g + 8) * 64, :]  # [512, C]
        m = dict(common)
        m["xT"] = xT_img[b]
        m["wq"] = _img_ct(wq_p.astype(BF16_NP), 512)
        m["wkv"] = _img_ct(wkv_p.astype(BF16_NP), 256)
        m["wo"] = _img_ct(wo_p.astype(BF16_NP), C)
        in_maps.append(m)
    return in_maps


_NC_CACHE = {}


def _get_nc(C, T, B=None):
    key = (C, T)
    if key not in _NC_CACHE:
        _NC_CACHE[key] = build_nc(C, T)
    return _NC_CACHE[key]


def run(x, Wq, Wk, Wv, Wo, trace=False):
    from concourse.bass_utils import run_bass_kernel_spmd

    B, T, C = x.shape
    nc = _get_nc(C, T)
    in_maps = make_inputs(x, Wq, Wk, Wv, Wo)
    try:
        res = run_bass_kernel_spmd(nc, in_maps, list(range(8)), trace=trace)
    except (ImportError, ModuleNotFoundError):
        res = run_bass_kernel_spmd(nc, in_maps, list(range(8)), trace=False)
    outs = []
    for b in range(B):
        acc = res.results[4 * b]["out"].astype(np.float32)
        for g in range(1, 4):
            acc = acc + res.results[4 * b + g]["out"].astype(np.float32)
        outs.append(acc)
    return np.stack(outs, 0), res


def kernel(x, Wq, Wk, Wv, Wo):
    out, _ = run(x, Wq, Wk, Wv, Wo, trace=False)
    return out


# revision 49
# speedup vs baseline: 1.0202x; 1.0202x over previous
"""GQA kernel for TRN2, 8 cores: DP2 (batch) x TP4 (KV-head pairs).

Core i: batch i//4, KV heads {2g, 2g+1} (g = i%4), Q heads 8g..8g+7.

Per core (B=1, T=2048, C=2048 local view):
  - All operands bf16 (host-converted); matmul accum fp32 in PSUM.
  - Q^T/K^T computed as [dims, tok] pair-tiles [128, 512]; RoPE on DVE
    (6 ops/tile) writes bf16 QT/KT. Q pairs are (q_j, q_j+4) so the two
    64-row halves use KV head A/B respectively (base-partition match).
  - V computed directly transposed: out[tok, vdim] (lhsT = x chunk).
  - Scores S^T[k, q] per 128-q subtile j, key tiles ki<=j only (fine
    causal granularity; no memsets). 8 heads share one [128, 1024] PSUM
    tile -> single exp per ki. Diagonal ki==j masked by one tri-multiply
    over all heads ([128,1024] with an 8x-replicated tri constant).
  - AV in [q, d] orientation: out[128 q, 64] per head, N=64 matmuls, plus
    N=1 ones-matmuls accumulating softmax denominators (PSUM [128, 8]).
    PSUM pending-zero is bank-granular, so each bank's accumulation group
    opens/closes exactly once per subtile.
  - Scores split in two 4-head groups of [128, 512] so PSUM fits
    (S 3-buf + AV + proj 2-buf + outproj + misc = 8 banks); all matmuls
    into one S tile share a tile_position (mixing 0/64 within one tile
    hangs the device).
  - Normalize: one DVE tensor_mul with a stride-0-broadcast reciprocal;
    transpose Y on PE into a bf16 view of the misc bank; out-projection
    accumulates [128, 512] C-chunks, copies spread over ACT/DVE.
  - Host sums the 4 bf16 partial outputs per batch.
"""

import sys

for p in ("/opt/trn_rl_repo", "/root/.axon_site/_ro/trn_rl_repo"):
    if p not in sys.path:
        sys.path.insert(0, p)

import numpy as np
import ml_dtypes
from contextlib import ExitStack

import concourse.bacc as bacc
import concourse.mybir as mybir
import concourse.tile as tile

F32 = mybir.dt.float32
BF16 = mybir.dt.bfloat16
BF16_NP = ml_dtypes.bfloat16

D = 64
ROPE_BASE = 10000.0
TCH = 512            # projection token chunk
EXPF = mybir.ActivationFunctionType.Exp
DEBUG_DUMP = False   # dump j=1 intermediates into `out` instead of results


def build_nc(C, T):
    CT = C // 128            # contraction tiles (16)
    NCH = T // TCH           # proj chunks (4)
    NJ = T // 128            # 128-token subtiles (16)
    JPC = TCH // 128         # subtiles per chunk (4)

    nc = bacc.Bacc("TRN2", target_bir_lowering=False, debug=False)

    xT = nc.dram_tensor("xT", [128, CT * T], BF16, kind="ExternalInput")
    wq = nc.dram_tensor("wq", [128, CT * 512], BF16, kind="ExternalInput")
    wkv = nc.dram_tensor("wkv", [128, CT * 256], BF16, kind="ExternalInput")
    wo = nc.dram_tensor("wo", [128, 4 * C], BF16, kind="ExternalInput")
    rqc = nc.dram_tensor("rqc", [128, T], BF16, kind="ExternalInput")
    rqs = nc.dram_tensor("rqs", [128, T], BF16, kind="ExternalInput")
    rkc = nc.dram_tensor("rkc", [128, T], BF16, kind="ExternalInput")
    rks = nc.dram_tensor("rks", [128, T], BF16, kind="ExternalInput")
    tri8 = nc.dram_tensor("tri8", [128, 1024], BF16, kind="ExternalInput")
    ident = nc.dram_tensor("ident", [128, 128], BF16, kind="ExternalInput")
    out = nc.dram_tensor("out", [T, C], BF16, kind="ExternalOutput")

    with tile.TileContext(nc) as tc, ExitStack() as ctx:
        # PSUM pools: creation order fixes bank layout; keep every pool's
        # per-partition size a multiple of 2048B so tiles stay bank-aligned.
        ps_s = ctx.enter_context(tc.tile_pool(name="ps_s", bufs=3, space="PSUM"))
        ps_av = ctx.enter_context(tc.tile_pool(name="ps_av", bufs=1, space="PSUM"))
        ps_pj = ctx.enter_context(tc.tile_pool(name="ps_pj", bufs=2, space="PSUM"))
        ps_o = ctx.enter_context(tc.tile_pool(name="ps_o", bufs=1, space="PSUM"))
        ps_m = ctx.enter_context(tc.tile_pool(name="ps_m", bufs=1, space="PSUM"))

        cst = ctx.enter_context(tc.tile_pool(name="cst", bufs=1))
        xcp = ctx.enter_context(tc.tile_pool(name="xcp", bufs=3))
        tmpp = ctx.enter_context(tc.tile_pool(name="tmpp", bufs=3))
        pp = ctx.enter_context(tc.tile_pool(name="pp", bufs=6))
        rcp = ctx.enter_context(tc.tile_pool(name="rcp", bufs=2))
        ysp = ctx.enter_context(tc.tile_pool(name="ysp", bufs=3))
        ytp = ctx.enter_context(tc.tile_pool(name="ytp", bufs=3))
        osp = ctx.enter_context(tc.tile_pool(name="osp", bufs=3))

        WQ = cst.tile([128, CT * 512], BF16, tag="WQ")
        WKV = cst.tile([128, CT * 256], BF16, tag="WKV")
        WO = cst.tile([128, 4 * C], BF16, tag="WO")
        QC = cst.tile([128, T], BF16, tag="QC")
        QS = cst.tile([128, T], BF16, tag="QS")
        KC = cst.tile([128, T], BF16, tag="KC")
        KS = cst.tile([128, T], BF16, tag="KS")
        TRI8 = cst.tile([128, 1024], BF16, tag="TRI8")
        ID = cst.tile([128, 128], BF16, tag="ID")
        ONES = cst.tile([128, 1], BF16, tag="ONES")
        QT = cst.tile([128, 4 * T], BF16, tag="QT")
        KT = cst.tile([128, T], BF16, tag="KT")
        VPA = cst.tile([128, NJ * 64], BF16, tag="VPA")
        VPB = cst.tile([128, NJ * 64], BF16, tag="VPB")
        # Persistent 1-bank PSUM scratch: softmax denominators (j-parity
        # rotated 8-col slots) + transpose staging (bf16 view of cols 16:272).
        MISC = ps_m.tile([128, 512], F32, tag="m")

        # DMA issue order sets DMA-device service order: first x half-chunk
        # and the weights that unblock the first projections, then tables,
        # then the rest.
        XCs = [xcp.tile([128, CT * TCH], BF16, tag="XC", name=f"XC{i}") for i in range(NCH)]
        QUAR = CT * TCH // 4
        # One need-ordered stream on SP (interleaving WKV/x quarters so the
        # K projection can start ~2us in), then the rest.
        WKVQ = CT * 256 // 4
        WQQ = CT * 512 // 4
        for qq in range(4):
            nc.sync.dma_start(WKV[:, qq * WKVQ:(qq + 1) * WKVQ],
                              wkv[:, qq * WKVQ:(qq + 1) * WKVQ])
            nc.sync.dma_start(XCs[0][:, qq * QUAR:(qq + 1) * QUAR],
                              xT[:, qq * QUAR:(qq + 1) * QUAR])
        nc.sync.dma_start(KC[:], rkc[:])
        nc.sync.dma_start(KS[:], rks[:])
        for qq in range(4):
            nc.sync.dma_start(WQ[:, qq * WQQ:(qq + 1) * WQQ],
                              wq[:, qq * WQQ:(qq + 1) * WQQ])
        nc.sync.dma_start(QC[:], rqc[:])
        nc.sync.dma_start(QS[:], rqs[:])
        nc.scalar.dma_start(TRI8[:], tri8[:])
        nc.scalar.dma_start(ID[:], ident[:])
        nc.sync.dma_start(XCs[1][:], xT[:, CT * TCH:2 * CT * TCH])
        nc.gpsimd.dma_start(WO[:], wo[:])
        nc.vector.memset(ONES[:], 1.0)

        def rope(ps, cosT, sinT, dst, tw0):
            """dst[128, 512] (bf16) = RoPE(ps[128, 512]) for a 2x64-row pair.
            One PSUM read (copy) frees the proj bank after ~0.7us instead of
            ~2.8us; the remaining ops are all-bf16 SBUF (2x DVE mode)."""
            psc = tmpp.tile([128, TCH], BF16, tag="psc")
            tco = tmpp.tile([128, TCH], BF16, tag="tco")
            tsi = tmpp.tile([128, TCH], BF16, tag="tsi")
            nc.vector.tensor_copy(psc[:], ps[:])
            nc.vector.tensor_mul(tco[:], psc[:], cosT[:, tw0:tw0 + TCH])
            for b0 in (0, 64):
                nc.vector.tensor_mul(
                    tsi[b0:b0 + 32, :], psc[b0 + 32:b0 + 64, :],
                    sinT[b0 + 32:b0 + 64, tw0:tw0 + TCH])
                nc.vector.tensor_mul(
                    tsi[b0 + 32:b0 + 64, :], psc[b0:b0 + 32, :],
                    sinT[b0:b0 + 32, tw0:tw0 + TCH])
            nc.vector.tensor_add(dst, tco[:], tsi[:])

        pend = {"work": None}

        def emit_outproj(n, pool=None, ptag="o"):
            """Emit up to n deferred out-projection co-units (4 MMs + copy),
            plus the output DMA when the unit completes."""
            w = pend["work"]
            if w is None:
                return
            j_, YT_, OSB_, co0 = w
            for co in range(co0, min(co0 + n, 4)):
                OPS = (pool or ps_o).tile([128, 512], F32, tag=ptag)
                for p in range(4):
                    nc.tensor.matmul(
                        OPS[:], YT_[:, p * 128:p * 128 + 128],
                        WO[:, p * C + co * 512:p * C + co * 512 + 512],
                        start=(p == 0), stop=(p == 3))
                if co < 2 and j_ < 8:
                    nc.scalar.copy(OSB_[:, co * 512:co * 512 + 512], OPS[:])
                else:
                    nc.vector.tensor_copy(OSB_[:, co * 512:co * 512 + 512], OPS[:])
            w[3] = min(co0 + n, 4)
            if w[3] == 4:
                nc.sync.dma_start(out[j_ * 128:(j_ + 1) * 128, :], OSB_[:])
                pend["work"] = None

        for tcl in range(NCH):
            XC = XCs[tcl]
            if tcl >= 2:
                nc.sync.dma_start(XC[:], xT[:, tcl * CT * TCH:(tcl + 1) * CT * TCH])
            tw0 = tcl * TCH
            # K projection (one [128,512] pair-tile: kv heads A/B)
            KPS = ps_pj.tile([128, TCH], F32, tag="pj")
            for ct in range(CT):
                nc.tensor.matmul(
                    KPS[:], WKV[:, ct * 256:ct * 256 + 128],
                    XC[:, ct * TCH:(ct + 1) * TCH],
                    start=(ct == 0), stop=(ct == CT - 1))
            rope(KPS, KC, KS, KT[:, tw0:tw0 + TCH], tw0)
            # Q projections (4 pair-tiles)
            for j in range(4):
                QPS = ps_pj.tile([128, TCH], F32, tag="pj")
                for ct in range(CT):
                    nc.tensor.matmul(
                        QPS[:], WQ[:, ct * 512 + j * 128:ct * 512 + j * 128 + 128],
                        XC[:, ct * TCH:(ct + 1) * TCH],
                        start=(ct == 0), stop=(ct == CT - 1))
                rope(QPS, QC, QS, QT[:, j * T + tw0:j * T + tw0 + TCH], tw0)
            # V projection: out[tok, vdim] per 128-token subtile
            VPS = ps_pj.tile([128, TCH], F32, tag="pj")
            for tt in range(JPC):
                for ct in range(CT):
                    nc.tensor.matmul(
                        VPS[:, tt * 128:tt * 128 + 128],
                        XC[:, ct * TCH + tt * 128:ct * TCH + tt * 128 + 128],
                        WKV[:, ct * 256 + 128:ct * 256 + 256],
                        start=(ct == 0), stop=(ct == CT - 1))
            for tt in range(JPC):
                kt = tcl * JPC + tt
                nc.vector.tensor_copy(VPA[:, kt * 64:kt * 64 + 64], VPS[:, tt * 128:tt * 128 + 64])
                nc.vector.tensor_copy(VPB[:, kt * 64:kt * 64 + 64], VPS[:, tt * 128 + 64:tt * 128 + 128])

            # attention for this chunk's four 128-q subtiles; out-projection
            # of subtile j is deferred and interleaved into attention j+1's
            # ki loop (fills PE during exp waits).
            for jj in range(JPC):
                j = tcl * JPC + jj
                AVP = ps_av.tile([128, 512], F32, tag="av")
                DEN = MISC[:, (j % 2) * 8:(j % 2) * 8 + 8]
                # Group A covers Q pairs 0-1 (heads 0,4,1,5), group B pairs
                # 2-3 -- so group A's whole ki loop can stream as soon as half
                # the Q projection of this chunk is done.
                # NOTE: start=True zeroes the whole 2KB PSUM bank (pending-
                # zero is bank-granular), so each bank's accumulation group
                # opens exactly once (first write) and closes on the last.
                for ki in range(j + 1):
                    for grp in range(2):
                        heads = [4 * grp, 4 * grp + 1, 4 * grp + 2, 4 * grp + 3]
                        S = ps_s.tile([128, 512], F32, tag="s")
                        for hh, h in enumerate(heads):
                            b0 = (h // 4) * 64
                            nc.tensor.matmul(
                                S[:, hh * 128:hh * 128 + 128],
                                KT[b0:b0 + 64, ki * 128:ki * 128 + 128],
                                QT[b0:b0 + 64, (h % 4) * T + j * 128:(h % 4) * T + j * 128 + 128],
                                start=True, stop=True)
                        P = pp.tile([128, 512], BF16, tag="P")
                        nc.scalar.activation(P[:], S[:], EXPF)
                        if ki == j:
                            nc.vector.tensor_mul(P[:], P[:], TRI8[:, 0:512])
                        for hh, h in enumerate(heads):
                            VP = VPA if h < 4 else VPB
                            nc.tensor.matmul(
                                AVP[:, h * 64:h * 64 + 64], P[:, hh * 128:hh * 128 + 128],
                                VP[:, ki * 64:ki * 64 + 64],
                                start=(grp == 0 and ki == 0 and hh == 0),
                                stop=(grp == 1 and ki == j and hh == 3),
                                skip_group_check=True)
                            nc.tensor.matmul(
                                DEN[:, h:h + 1], P[:, hh * 128:hh * 128 + 128], ONES[:],
                                start=(grp == 0 and ki == 0 and hh == 0),
                                stop=(grp == 1 and ki == j and hh == 3),
                                skip_group_check=True)

                RC = rcp.tile([128, 8], F32, tag="RC")
                nc.vector.reciprocal(RC[:], DEN)
                YSB = ysp.tile([128, 512], BF16, tag="YSB")
                nc.vector.tensor_mul(
                    YSB[:], AVP[:, 0:512],
                    RC[:, :, None].broadcast_to([128, 8, 64]))
                YTt = ytp.tile([128, 512], BF16, tag="YT")
                TPSv = MISC[:, 16:272].bitcast(BF16)
                for p in range(4):
                    nc.tensor.transpose(
                        TPSv[:, p * 128:p * 128 + 128],
                        YSB[:, p * 128:p * 128 + 128], ID[:])
                nc.vector.tensor_copy(YTt[:], TPSv[:])
                OSBn = osp.tile([128, C], BF16, tag="OSB", name=f"OSB{j}")
                pend["work"] = [j, YTt, OSBn, 0]
                emit_outproj(4)

        emit_outproj(4, pool=ps_pj, ptag="pj")

    nc.compile()
    return nc


def rope_tables(T, scale):
    inv = 1.0 / (ROPE_BASE ** (np.arange(0, D, 2, dtype=np.float32) / D))
    t = np.arange(T, dtype=np.float32)
    freqs = np.outer(t, inv)  # [T, 32]
    emb = np.concatenate([freqs, freqs], -1)  # [T, 64]
    cos = np.cos(emb).T.astype(np.float32) * scale  # [64, T]
    sin = np.sin(emb).T.astype(np.float32) * scale
    sinX = np.empty((64, T), np.float32)
    sinX[0:32] = sin[32:64]
    sinX[32:64] = -sin[0:32]
    cos2 = np.concatenate([cos, cos], 0)
    sin2 = np.concatenate([sinX, sinX], 0)
    return np.ascontiguousarray(cos2), np.ascontiguousarray(sin2)


def _img_ct(w, cols):
    """[C, cols] -> [128, CT*cols] image: img[p, ct*cols + c] = w[ct*128+p, c]."""
    C = w.shape[0]
    CT = C // 128
    return np.ascontiguousarray(
        w.reshape(CT, 128, cols).transpose(1, 0, 2).reshape(128, CT * cols))


def make_inputs(x, Wq, Wk, Wv, Wo):
    B, T, C = x.shape
    CT = C // 128
    qc, qs = rope_tables(T, 1.0 / np.sqrt(D).astype(np.float32))
    kc, ks = rope_tables(T, 1.0)
    tri = np.triu(np.ones((128, 128), np.float32)).astype(BF16_NP)
    tri8 = np.ascontiguousarray(np.tile(tri, (1, 8)))
    common = {
        "rqc": qc.astype(BF16_NP), "rqs": qs.astype(BF16_NP),
        "rkc": kc.astype(BF16_NP), "rks": ks.astype(BF16_NP),
        "tri8": tri8,
        "ident": np.eye(128, dtype=np.float32).astype(BF16_NP),
    }
    xT_img = []
    for b in range(B):
        # [128, NCH * CT * TCH]: chunk-major, then ct, then token
        xb = x[b].astype(BF16_NP)  # [T, C]
        img = (xb.reshape(T // TCH, TCH, CT, 128)
               .transpose(3, 0, 2, 1)   # [128, NCH, CT, TCH]
               .reshape(128, -1))
        xT_img.append(np.ascontiguousarray(img))
    in_maps = []
    for i in range(8):
        b, g = divmod(i, 4)
        # Q head pairing: pair j = (local head j, local head j+4)
        qcols = []
        for j in range(4):
            qcols.append(Wq[:, (8 * g + j) * 64:(8 * g + j) * 64 + 64])
            qcols.append(Wq[:, (8 * g + j + 4) * 64:(8 * g + j + 4) * 64 + 64])
        wq_p = np.concatenate(qcols, 1)  # [C, 512] pair-major
        wkv_p = np.concatenate(
            [Wk[:, (2 * g) * 64:(2 * g + 2) * 64],
             Wv[:, (2 * g) * 64:(2 * g + 2) * 64]], 1)  # [C, 256]
        # Y/YT dim order comes out as natural local heads 0..7, so Wo rows
        # are the plain local slice.
        wo_p = Wo[8 * g * 64:(8 * g + 8) * 64, :]  # [512, C]
        m = dict(common)
        m["xT"] = xT_img[b]
        m["wq"] = _img_ct(wq_p.astype(BF16_NP), 512)
        m["wkv"] = _img_ct(wkv_p.astype(BF16_NP), 256)
        m["wo"] = _img_ct(wo_p.astype(BF16_NP), C)
        in_maps.append(m)
    return in_maps


_NC_CACHE = {}


def _get_nc(C, T, B=None):
    key = (C, T)
    if key not in _NC_CACHE:
        _NC_CACHE[key] = build_nc(C, T)
    return _NC_CACHE[key]


def run(x, Wq, Wk, Wv, Wo, trace=False):
    from concourse.bass_utils import run_bass_kernel_spmd

    B, T, C = x.shape
    nc = _get_nc(C, T)
    in_maps = make_inputs(x, Wq, Wk, Wv, Wo)
    try:
        res = run_bass_kernel_spmd(nc, in_maps, list(range(8)), trace=trace)
    except (ImportError, ModuleNotFoundError):
        res = run_bass_kernel_spmd(nc, in_maps, list(range(8)), trace=False)
    outs = []
    for b in range(B):
        acc = res.results[4 * b]["out"].astype(np.float32)
        for g in range(1, 4):
            acc = acc + res.results[4 * b + g]["out"].astype(np.float32)
        outs.append(acc)
    return np.stack(outs, 0), res


def kernel(x, Wq, Wk, Wv, Wo):
    out, _ = run(x, Wq, Wk, Wv, Wo, trace=False)
    return out


# revision 51
# speedup vs baseline: 1.0204x; 1.0003x over previous
"""GQA kernel for TRN2, 8 cores: DP2 (batch) x TP4 (KV-head pairs).

Core i: batch i//4, KV heads {2g, 2g+1} (g = i%4), Q heads 8g..8g+7.

Per core (B=1, T=2048, C=2048 local view):
  - All operands bf16 (host-converted); matmul accum fp32 in PSUM.
  - Q^T/K^T computed as [dims, tok] pair-tiles [128, 512]; RoPE on DVE
    (6 ops/tile) writes bf16 QT/KT. Q pairs are (q_j, q_j+4) so the two
    64-row halves use KV head A/B respectively (base-partition match).
  - V computed directly transposed: out[tok, vdim] (lhsT = x chunk).
  - Scores S^T[k, q] per 128-q subtile j, key tiles ki<=j only (fine
    causal granularity; no memsets). 8 heads share one [128, 1024] PSUM
    tile -> single exp per ki. Diagonal ki==j masked by one tri-multiply
    over all heads ([128,1024] with an 8x-replicated tri constant).
  - AV in [q, d] orientation: out[128 q, 64] per head, N=64 matmuls, plus
    N=1 ones-matmuls accumulating softmax denominators (PSUM [128, 8]).
    PSUM pending-zero is bank-granular, so each bank's accumulation group
    opens/closes exactly once per subtile.
  - Scores split in two 4-head groups of [128, 512] so PSUM fits
    (S 3-buf + AV + proj 2-buf + outproj + misc = 8 banks); all matmuls
    into one S tile share a tile_position (mixing 0/64 within one tile
    hangs the device).
  - Normalize: one DVE tensor_mul with a stride-0-broadcast reciprocal;
    transpose Y on PE into a bf16 view of the misc bank; out-projection
    accumulates [128, 512] C-chunks, copies spread over ACT/DVE.
  - Host sums the 4 bf16 partial outputs per batch.
"""

import sys

for p in ("/opt/trn_rl_repo", "/root/.axon_site/_ro/trn_rl_repo"):
    if p not in sys.path:
        sys.path.insert(0, p)

import numpy as np
import ml_dtypes
from contextlib import ExitStack

import concourse.bacc as bacc
import concourse.mybir as mybir
import concourse.tile as tile

F32 = mybir.dt.float32
BF16 = mybir.dt.bfloat16
BF16_NP = ml_dtypes.bfloat16

D = 64
ROPE_BASE = 10000.0
TCH = 512            # projection token chunk
EXPF = mybir.ActivationFunctionType.Exp
DEBUG_DUMP = False   # dump j=1 intermediates into `out` instead of results


def build_nc(C, T):
    CT = C // 128            # contraction tiles (16)
    NCH = T // TCH           # proj chunks (4)
    NJ = T // 128            # 128-token subtiles (16)
    JPC = TCH // 128         # subtiles per chunk (4)

    nc = bacc.Bacc("TRN2", target_bir_lowering=False, debug=False)

    xT = nc.dram_tensor("xT", [128, CT * T], BF16, kind="ExternalInput")
    wq = nc.dram_tensor("wq", [128, CT * 512], BF16, kind="ExternalInput")
    wkv = nc.dram_tensor("wkv", [128, CT * 256], BF16, kind="ExternalInput")
    wo = nc.dram_tensor("wo", [128, 4 * C], BF16, kind="ExternalInput")
    rqc = nc.dram_tensor("rqc", [128, T], BF16, kind="ExternalInput")
    rqs = nc.dram_tensor("rqs", [128, T], BF16, kind="ExternalInput")
    rkc = nc.dram_tensor("rkc", [128, T], BF16, kind="ExternalInput")
    rks = nc.dram_tensor("rks", [128, T], BF16, kind="ExternalInput")
    tri8 = nc.dram_tensor("tri8", [128, 1024], BF16, kind="ExternalInput")
    ident = nc.dram_tensor("ident", [128, 128], BF16, kind="ExternalInput")
    out = nc.dram_tensor("out", [T, C], BF16, kind="ExternalOutput")

    with tile.TileContext(nc) as tc, ExitStack() as ctx:
        # PSUM pools: creation order fixes bank layout; keep every pool's
        # per-partition size a multiple of 2048B so tiles stay bank-aligned.
        ps_s = ctx.enter_context(tc.tile_pool(name="ps_s", bufs=3, space="PSUM"))
        ps_av = ctx.enter_context(tc.tile_pool(name="ps_av", bufs=1, space="PSUM"))
        ps_pj = ctx.enter_context(tc.tile_pool(name="ps_pj", bufs=2, space="PSUM"))
        ps_o = ctx.enter_context(tc.tile_pool(name="ps_o", bufs=1, space="PSUM"))
        ps_m = ctx.enter_context(tc.tile_pool(name="ps_m", bufs=1, space="PSUM"))

        cst = ctx.enter_context(tc.tile_pool(name="cst", bufs=1))
        xcp = ctx.enter_context(tc.tile_pool(name="xcp", bufs=4))
        tmpp = ctx.enter_context(tc.tile_pool(name="tmpp", bufs=3))
        pp = ctx.enter_context(tc.tile_pool(name="pp", bufs=6))
        rcp = ctx.enter_context(tc.tile_pool(name="rcp", bufs=2))
        ysp = ctx.enter_context(tc.tile_pool(name="ysp", bufs=3))
        ytp = ctx.enter_context(tc.tile_pool(name="ytp", bufs=3))
        osp = ctx.enter_context(tc.tile_pool(name="osp", bufs=3))

        WQ = cst.tile([128, CT * 512], BF16, tag="WQ")
        WKV = cst.tile([128, CT * 256], BF16, tag="WKV")
        WO = cst.tile([128, 4 * C], BF16, tag="WO")
        QC = cst.tile([128, T], BF16, tag="QC")
        QS = cst.tile([128, T], BF16, tag="QS")
        KC = cst.tile([128, T], BF16, tag="KC")
        KS = cst.tile([128, T], BF16, tag="KS")
        TRI8 = cst.tile([128, 1024], BF16, tag="TRI8")
        ID = cst.tile([128, 128], BF16, tag="ID")
        ONES = cst.tile([128, 1], BF16, tag="ONES")
        QT = cst.tile([128, 4 * T], BF16, tag="QT")
        KT = cst.tile([128, T], BF16, tag="KT")
        VPA = cst.tile([128, NJ * 64], BF16, tag="VPA")
        VPB = cst.tile([128, NJ * 64], BF16, tag="VPB")
        # Persistent 1-bank PSUM scratch: softmax denominators (j-parity
        # rotated 8-col slots) + transpose staging (bf16 view of cols 16:272).
        MISC = ps_m.tile([128, 512], F32, tag="m")

        # DMA issue order sets DMA-device service order: first x half-chunk
        # and the weights that unblock the first projections, then tables,
        # then the rest.
        XCs = [xcp.tile([128, CT * TCH], BF16, tag="XC", name=f"XC{i}") for i in range(NCH)]
        QUAR = CT * TCH // 4
        # One need-ordered stream on SP (interleaving WKV/x quarters so the
        # K projection can start ~2us in), then the rest.
        WKVQ = CT * 256 // 4
        WQQ = CT * 512 // 4
        for qq in range(4):
            nc.sync.dma_start(WKV[:, qq * WKVQ:(qq + 1) * WKVQ],
                              wkv[:, qq * WKVQ:(qq + 1) * WKVQ])
            nc.sync.dma_start(XCs[0][:, qq * QUAR:(qq + 1) * QUAR],
                              xT[:, qq * QUAR:(qq + 1) * QUAR])
        nc.sync.dma_start(KC[:], rkc[:])
        nc.sync.dma_start(KS[:], rks[:])
        for qq in range(4):
            nc.sync.dma_start(WQ[:, qq * WQQ:(qq + 1) * WQQ],
                              wq[:, qq * WQQ:(qq + 1) * WQQ])
        nc.sync.dma_start(QC[:], rqc[:])
        nc.sync.dma_start(QS[:], rqs[:])
        nc.scalar.dma_start(TRI8[:], tri8[:])
        nc.scalar.dma_start(ID[:], ident[:])
        nc.sync.dma_start(XCs[1][:], xT[:, CT * TCH:2 * CT * TCH])
        nc.sync.dma_start(XCs[2][:], xT[:, 2 * CT * TCH:3 * CT * TCH])
        nc.sync.dma_start(XCs[3][:], xT[:, 3 * CT * TCH:4 * CT * TCH])
        nc.gpsimd.dma_start(WO[:], wo[:])
        nc.vector.memset(ONES[:], 1.0)

        def rope(ps, cosT, sinT, dst, tw0):
            """dst[128, 512] (bf16) = RoPE(ps[128, 512]) for a 2x64-row pair.
            One PSUM read (copy) frees the proj bank after ~0.7us instead of
            ~2.8us; the remaining ops are all-bf16 SBUF (2x DVE mode)."""
            psc = tmpp.tile([128, TCH], BF16, tag="psc")
            tco = tmpp.tile([128, TCH], BF16, tag="tco")
            tsi = tmpp.tile([128, TCH], BF16, tag="tsi")
            nc.vector.tensor_copy(psc[:], ps[:])
            nc.vector.tensor_mul(tco[:], psc[:], cosT[:, tw0:tw0 + TCH])
            for b0 in (0, 64):
                nc.vector.tensor_mul(
                    tsi[b0:b0 + 32, :], psc[b0 + 32:b0 + 64, :],
                    sinT[b0 + 32:b0 + 64, tw0:tw0 + TCH])
                nc.vector.tensor_mul(
                    tsi[b0 + 32:b0 + 64, :], psc[b0:b0 + 32, :],
                    sinT[b0:b0 + 32, tw0:tw0 + TCH])
            nc.vector.tensor_add(dst, tco[:], tsi[:])

        pend = {"work": None}

        def emit_outproj(n):
            """Emit up to n deferred out-projection co-units (4 MMs + copy),
            plus the output DMA when the unit completes."""
            w = pend["work"]
            if w is None:
                return
            j_, YT_, OSB_, co0 = w
            for co in range(co0, min(co0 + n, 4)):
                OPS = ps_o.tile([128, 512], F32, tag="o")
                for p in range(4):
                    nc.tensor.matmul(
                        OPS[:], YT_[:, p * 128:p * 128 + 128],
                        WO[:, p * C + co * 512:p * C + co * 512 + 512],
                        start=(p == 0), stop=(p == 3))
                if co == 0 and j_ < 8:
                    nc.scalar.copy(OSB_[:, co * 512:co * 512 + 512], OPS[:])
                else:
                    nc.vector.tensor_copy(OSB_[:, co * 512:co * 512 + 512], OPS[:])
            w[3] = min(co0 + n, 4)
            if w[3] == 4:
                nc.sync.dma_start(out[j_ * 128:(j_ + 1) * 128, :], OSB_[:])
                pend["work"] = None

        for tcl in range(NCH):
            XC = XCs[tcl]
            tw0 = tcl * TCH
            # K projection (one [128,512] pair-tile: kv heads A/B)
            KPS = ps_pj.tile([128, TCH], F32, tag="pj")
            for ct in range(CT):
                nc.tensor.matmul(
                    KPS[:], WKV[:, ct * 256:ct * 256 + 128],
                    XC[:, ct * TCH:(ct + 1) * TCH],
                    start=(ct == 0), stop=(ct == CT - 1))
            rope(KPS, KC, KS, KT[:, tw0:tw0 + TCH], tw0)
            # Q projections (4 pair-tiles)
            for j in range(4):
                QPS = ps_pj.tile([128, TCH], F32, tag="pj")
                for ct in range(CT):
                    nc.tensor.matmul(
                        QPS[:], WQ[:, ct * 512 + j * 128:ct * 512 + j * 128 + 128],
                        XC[:, ct * TCH:(ct + 1) * TCH],
                        start=(ct == 0), stop=(ct == CT - 1))
                rope(QPS, QC, QS, QT[:, j * T + tw0:j * T + tw0 + TCH], tw0)
            # V projection: out[tok, vdim] per 128-token subtile
            VPS = ps_pj.tile([128, TCH], F32, tag="pj")
            for tt in range(JPC):
                for ct in range(CT):
                    nc.tensor.matmul(
                        VPS[:, tt * 128:tt * 128 + 128],
                        XC[:, ct * TCH + tt * 128:ct * TCH + tt * 128 + 128],
                        WKV[:, ct * 256 + 128:ct * 256 + 256],
                        start=(ct == 0), stop=(ct == CT - 1))
            for tt in range(JPC):
                kt = tcl * JPC + tt
                nc.vector.tensor_copy(VPA[:, kt * 64:kt * 64 + 64], VPS[:, tt * 128:tt * 128 + 64])
                nc.vector.tensor_copy(VPB[:, kt * 64:kt * 64 + 64], VPS[:, tt * 128 + 64:tt * 128 + 128])

            # attention for this chunk's four 128-q subtiles; out-projection
            # of subtile j is deferred and interleaved into attention j+1's
            # ki loop (fills PE during exp waits).
            for jj in range(JPC):
                j = tcl * JPC + jj
                AVP = ps_av.tile([128, 512], F32, tag="av")
                DEN = MISC[:, (j % 2) * 8:(j % 2) * 8 + 8]
                # Group A covers Q pairs 0-1 (heads 0,4,1,5), group B pairs
                # 2-3 -- so group A's whole ki loop can stream as soon as half
                # the Q projection of this chunk is done.
                # NOTE: start=True zeroes the whole 2KB PSUM bank (pending-
                # zero is bank-granular), so each bank's accumulation group
                # opens exactly once (first write) and closes on the last.
                for ki in range(j + 1):
                    for grp in range(2):
                        heads = [4 * grp, 4 * grp + 1, 4 * grp + 2, 4 * grp + 3]
                        S = ps_s.tile([128, 512], F32, tag="s")
                        for hh, h in enumerate(heads):
                            b0 = (h // 4) * 64
                            nc.tensor.matmul(
                                S[:, hh * 128:hh * 128 + 128],
                                KT[b0:b0 + 64, ki * 128:ki * 128 + 128],
                                QT[b0:b0 + 64, (h % 4) * T + j * 128:(h % 4) * T + j * 128 + 128],
                                start=True, stop=True)
                        P = pp.tile([128, 512], BF16, tag="P")
                        nc.scalar.activation(P[:], S[:], EXPF)
                        if ki == j:
                            nc.vector.tensor_mul(P[:], P[:], TRI8[:, 0:512])
                        for hh, h in enumerate(heads):
                            VP = VPA if h < 4 else VPB
                            nc.tensor.matmul(
                                AVP[:, h * 64:h * 64 + 64], P[:, hh * 128:hh * 128 + 128],
                                VP[:, ki * 64:ki * 64 + 64],
                                start=(grp == 0 and ki == 0 and hh == 0),
                                stop=(grp == 1 and ki == j and hh == 3),
                                skip_group_check=True)
                            nc.tensor.matmul(
                                DEN[:, h:h + 1], P[:, hh * 128:hh * 128 + 128], ONES[:],
                                start=(grp == 0 and ki == 0 and hh == 0),
                                stop=(grp == 1 and ki == j and hh == 3),
                                skip_group_check=True)

                RC = rcp.tile([128, 8], F32, tag="RC")
                nc.vector.reciprocal(RC[:], DEN)
                YSB = ysp.tile([128, 512], BF16, tag="YSB")
                nc.vector.tensor_mul(
                    YSB[:], AVP[:, 0:512],
                    RC[:, :, None].broadcast_to([128, 8, 64]))
                YTt = ytp.tile([128, 512], BF16, tag="YT")
                TPSv = MISC[:, 16:272].bitcast(BF16)
                for p in range(4):
                    nc.tensor.transpose(
                        TPSv[:, p * 128:p * 128 + 128],
                        YSB[:, p * 128:p * 128 + 128], ID[:])
                nc.vector.tensor_copy(YTt[:], TPSv[:])
                OSBn = osp.tile([128, C], BF16, tag="OSB", name=f"OSB{j}")
                pend["work"] = [j, YTt, OSBn, 0]
                emit_outproj(4)

        emit_outproj(4)

    nc.compile()
    return nc


def rope_tables(T, scale):
    inv = 1.0 / (ROPE_BASE ** (np.arange(0, D, 2, dtype=np.float32) / D))
    t = np.arange(T, dtype=np.float32)
    freqs = np.outer(t, inv)  # [T, 32]
    emb = np.concatenate([freqs, freqs], -1)  # [T, 64]
    cos = np.cos(emb).T.astype(np.float32) * scale  # [64, T]
    sin = np.sin(emb).T.astype(np.float32) * scale
    sinX = np.empty((64, T), np.float32)
    sinX[0:32] = sin[32:64]
    sinX[32:64] = -sin[0:32]
    cos2 = np.concatenate([cos, cos], 0)
    sin2 = np.concatenate([sinX, sinX], 0)
    return np.ascontiguousarray(cos2), np.ascontiguousarray(sin2)


def _img_ct(w, cols):
    """[C, cols] -> [128, CT*cols] image: img[p, ct*cols + c] = w[ct*128+p, c]."""
    C = w.shape[0]
    CT = C // 128
    return np.ascontiguousarray(
        w.reshape(CT, 128, cols).transpose(1, 0, 2).reshape(128, CT * cols))


def make_inputs(x, Wq, Wk, Wv, Wo):
    B, T, C = x.shape
    CT = C // 128
    qc, qs = rope_tables(T, 1.0 / np.sqrt(D).astype(np.float32))
    kc, ks = rope_tables(T, 1.0)
    tri = np.triu(np.ones((128, 128), np.float32)).astype(BF16_NP)
    tri8 = np.ascontiguousarray(np.tile(tri, (1, 8)))
    common = {
        "rqc": qc.astype(BF16_NP), "rqs": qs.astype(BF16_NP),
        "rkc": kc.astype(BF16_NP), "rks": ks.astype(BF16_NP),
        "tri8": tri8,
        "ident": np.eye(128, dtype=np.float32).astype(BF16_NP),
    }
    xT_img = []
    for b in range(B):
        # [128, NCH * CT * TCH]: chunk-major, then ct, then token
        xb = x[b].astype(BF16_NP)  # [T, C]
        img = (xb.reshape(T // TCH, TCH, CT, 128)
               .transpose(3, 0, 2, 1)   # [128, NCH, CT, TCH]
               .reshape(128, -1))
        xT_img.append(np.ascontiguousarray(img))
    in_maps = []
    for i in range(8):
        b, g = divmod(i, 4)
        # Q head pairing: pair j = (local head j, local head j+4)
        qcols = []
        for j in range(4):
            qcols.append(Wq[:, (8 * g + j) * 64:(8 * g + j) * 64 + 64])
            qcols.append(Wq[:, (8 * g + j + 4) * 64:(8 * g + j + 4) * 64 + 64])
        wq_p = np.concatenate(qcols, 1)  # [C, 512] pair-major
        wkv_p = np.concatenate(
            [Wk[:, (2 * g) * 64:(2 * g + 2) * 64],
             Wv[:, (2 * g) * 64:(2 * g + 2) * 64]], 1)  # [C, 256]
        # Y/YT dim order comes out as natural local heads 0..7, so Wo rows
        # are the plain local slice.
        wo_p = Wo[8 * g * 64:(8 * g + 8) * 64, :]  # [512, C]
        m = dict(common)
        m["xT"] = xT_img[b]
        m["wq"] = _img_ct(wq_p.astype(BF16_NP), 512)
        m["wkv"] = _img_ct(wkv_p.astype(BF16_NP), 256)
        m["wo"] = _img_ct(wo_p.astype(BF16_NP), C)
        in_maps.append(m)
    return in_maps


_NC_CACHE = {}


def _get_nc(C, T, B=None):
    key = (C, T)
    if key not in _NC_CACHE:
        _NC_CACHE[key] = build_nc(C, T)
    return _NC_CACHE[key]


def run(x, Wq, Wk, Wv, Wo, trace=False):
    from concourse.bass_utils import run_bass_kernel_spmd

    B, T, C = x.shape
    nc = _get_nc(C, T)
    in_maps = make_inputs(x, Wq, Wk, Wv, Wo)
    try:
        res = run_bass_kernel_spmd(nc, in_maps, list(range(8)), trace=trace)
    except (ImportError, ModuleNotFoundError):
        res = run_bass_kernel_spmd(nc, in_maps, list(range(8)), trace=False)
    outs = []
    for b in range(B):
        acc = res.results[4 * b]["out"].astype(np.float32)
        for g in range(1, 4):
            acc = acc + res.results[4 * b + g]["out"].astype(np.float32)
        outs.append(acc)
    return np.stack(outs, 0), res


def kernel(x, Wq, Wk, Wv, Wo):
    out, _ = run(x, Wq, Wk, Wv, Wo, trace=False)
    return out
